# revision 25
# baseline (speedup 1.0000x reference)
"""Trainium2 Bass kernel for nn_Attention_20461224198682.

Multi-head attention (B=64, N=196, C=768, H=12, D=64) with relative position
bias and key masking. Data-parallel over batch across 8 NeuronCores (8
batches/core). All device compute in a transposed layout (feature dim on
partitions) so no on-device transposes are needed:

  qkv^T[o,t]  = Wqkv^T-as-lhsT @ x^T    (q columns pre-scaled by D^-0.5;
                                         x^T / W^T / bf16 casts done on host)
  s^T[m,n]    = k^T-as-lhsT @ q^T       (keys m on partitions; both m-chunks
                                         land in one [98,392] PSUM bank)
  p[m,n]      = exp(s^T) * exp(bias)^T  (rpe bias exponentiated on host,
                                         applied multiplicatively post-exp)
  o^T[d,n]    = v-as-lhsT @ p           (v rows are pre-scaled by the key
                                         mask 0/1 - this also masks the ones
                                         column, so row 64 of o^T is exactly
                                         the masked softmax denominator)
  o^T[0:64]  /= denom                   (approx reciprocal straight from the
                                         PSUM denominator row + GpSimd
                                         partition_broadcast + DVE multiply)
  out^T[o2,t] = Wproj^T-as-lhsT @ concat_h o^T   (+ proj bias, which also
                                         absorbs the v bias: softmax rows
                                         sum to 1, so +vb is exact)

Matmuls run in bf16 (f32 PSUM accumulation); softmax path in f32. Inputs are
staged in DRAM as a few large contiguous blobs so each load wave is a single
DMA instruction (descriptor generation on the sync/scalar queues was the
startup bottleneck). Emission order streams attention behind the q/k
projection chains so PE, ACT, DVE and GpSimd overlap from a few us on.
"""
import numpy as np
import ml_dtypes

B, N, C, H = 64, 196, 768, 12
D = C // H
SCALE = D ** -0.5
TABLE = 729
NCORES = 8
NB = B // NCORES          # batches per core
TOK = NB * N              # tokens per core
MCH = [(0, 98), (98, 98)]   # key-token chunks within a batch
BF16 = ml_dtypes.bfloat16

# ---- wq blob column layout (wave order, shared host/device) ----
# j-major waves: A1 (q0,k0) | V (v per c) | A2 (q1,k1) | B2..B5 (qj,kj pairs)
_A1, _V0, _A2, _B0 = 0, 1536, 6144, 7680
WQ_COLS = 13824


def _qk_col(c, j):
    """column offset of the 128-wide block for q/k output j (0-11) chunk c"""
    if j == 0:
        return _A1 + c * 128
    if j == 6:
        return _A1 + 768 + c * 128
    if j == 1:
        return _A2 + c * 128
    if j == 7:
        return _A2 + 768 + c * 128
    if 2 <= j <= 5:
        return _B0 + (j - 2) * 1536 + c * 128
    return _B0 + (j - 8) * 1536 + 768 + c * 128


def _v_col(c):
    return _V0 + c * 768


_cache = {}


def _build_nc():
    import concourse.bass as bass
    import concourse.tile as tile
    from concourse import bacc, mybir

    f32 = mybir.dt.float32
    bf16 = mybir.dt.bfloat16
    AF = mybir.ActivationFunctionType
    AOT = mybir.AluOpType

    nc = bacc.Bacc()
    xt_d = nc.declare_dram_parameter("xt", [128, 4 * 2352], bf16, isOutput=False)
    wq_d = nc.declare_dram_parameter("wq", [128, WQ_COLS], bf16, isOutput=False)
    wp_d = nc.declare_dram_parameter("wp", [128, 4608], bf16, isOutput=False)
    biasT_d = nc.declare_dram_parameter("biasT", [128, H * 2 * N], bf16, isOutput=False)
    pars_d = nc.declare_dram_parameter("pars", [128, 34], f32, isOutput=False)
    out_d = nc.declare_dram_parameter("out", [C, TOK], bf16, isOutput=True)

    with tile.TileContext(nc) as tc:
        from contextlib import ExitStack
        with ExitStack() as ctx:
            p_w = ctx.enter_context(tc.tile_pool(name="w", bufs=1))
            p_qk = ctx.enter_context(tc.tile_pool(name="qk", bufs=48))
            p_vx = ctx.enter_context(tc.tile_pool(name="vx", bufs=16))
            p_small = ctx.enter_context(tc.tile_pool(name="small", bufs=8))
            p_sm = ctx.enter_context(tc.tile_pool(name="sm", bufs=6))
            p_cc = ctx.enter_context(tc.tile_pool(name="cc", bufs=12))
            p_ot = ctx.enter_context(tc.tile_pool(name="ot", bufs=4))
            pp = ctx.enter_context(tc.tile_pool(name="psum", bufs=8, space="PSUM"))

            # ---- persistent inputs: few large wave DMAs ----
            xt = p_w.tile([128, 4 * 2352], bf16, tag="xt")
            wq = p_w.tile([128, WQ_COLS], bf16, tag="wq")
            wp = p_w.tile([128, 4608], bf16, tag="wp")
            bias_t = p_w.tile([128, H * 2 * N], bf16, tag="biasT")
            pars = p_small.tile([128, 34], f32, tag="pars")
            ones12 = p_small.tile([128, 12], bf16, tag="ones12")

            maskp = pars[:, 0:16]

            # sync queue: first q/k wave + xt chunk-0 half + B waves;
            # scalar queue: the other xt half, pars, v, bias, rest of xt.
            # The first att step needs A1 + both xt0 halves; per-j B waves
            # unblock qkproj(j) without waiting for the whole weight image.
            nc.sync.dma_start(wq[:, _A1:_V0], wq_d[:, _A1:_V0])
            nc.scalar.dma_start(xt[:, 0:1176], xt_d[:, 0:1176])
            nc.sync.dma_start(xt[:, 1176:2352], xt_d[:, 1176:2352])
            nc.scalar.dma_start(pars[:], pars_d[:])
            # A2 + v weights ride the gpsimd SWDGE ring - a third DMA
            # channel, so neither queues behind the bias table or B waves
            nc.gpsimd.dma_start(wq[:, _A2:_B0], wq_d[:, _A2:_B0])
            nc.gpsimd.dma_start(wq[:, _V0:_A2], wq_d[:, _V0:_A2])
            nc.scalar.dma_start(bias_t[:, 0:2352], biasT_d[:, 0:2352])
            nc.sync.dma_start(wq[:, _B0:_B0 + 1536], wq_d[:, _B0:_B0 + 1536])
            nc.sync.dma_start(wq[:, _B0 + 1536:_B0 + 3072],
                              wq_d[:, _B0 + 1536:_B0 + 3072])
            nc.scalar.dma_start(bias_t[:, 2352:4704], biasT_d[:, 2352:4704])
            nc.sync.dma_start(wq[:, _B0 + 3072:_B0 + 4608],
                              wq_d[:, _B0 + 3072:_B0 + 4608])
            nc.scalar.dma_start(xt[:, 2352:4704], xt_d[:, 2352:4704])
            nc.sync.dma_start(wq[:, _B0 + 4608:WQ_COLS],
                              wq_d[:, _B0 + 4608:WQ_COLS])
            nc.scalar.dma_start(xt[:, 4704:7056], xt_d[:, 4704:7056])
            nc.sync.dma_start(wp[:], wp_d[:])
            nc.scalar.dma_start(xt[:, 7056:9408], xt_d[:, 7056:9408])
            nc.vector.memset(ones12[:], 1.0)

            def xt_ap(c, t0, w):
                # token range [t0, t0+w) must lie within one 392-token chunk
                chi, tl = t0 // 392, t0 % 392
                base = chi * 2352 + c * 392 + tl
                return xt[:, base:base + w]

            # ---- work units ----
            vx = {}
            qk = [[None] * 4 for _ in range(12)]
            cc = {}

            def vproj(b, mci):
                mc, msz = MCH[mci]
                vt = p_vx.tile([128, H * 65], bf16, tag="vx", bufs=16,
                               name=f"vx{b}_{mci}")
                ones_cols = vt[:, :].rearrange("p (h e) -> p h e", e=65)[:, :, 64:65]
                nc.scalar.activation(
                    ones_cols, ones12[:, :].rearrange("p (h e) -> p h e", e=1),
                    AF.Copy, scale=maskp[:, b * 2 + mci:b * 2 + mci + 1])
                for o0, hoff in ((0, 0), (384, 6)):
                    ps = pp.tile([128, 392], f32, tag="pj", bufs=3, name="psv")
                    for c in range(6):
                        nc.tensor.matmul(
                            ps[:msz, :384],
                            xt_ap(c, b * N + mc, msz),
                            wq[:, _v_col(c) + o0: _v_col(c) + o0 + 384],
                            start=(c == 0), stop=(c == 5),
                        )
                    dst = vt[:msz, hoff * 65:(hoff + 6) * 65].rearrange(
                        "p (h e) -> p h e", e=65)[:, :, 0:64]
                    src = ps[:msz, :384].rearrange("p (h e) -> p h e", e=64)
                    if hoff == 0:
                        nc.scalar.activation(
                            dst, src, AF.Copy,
                            scale=maskp[:msz, b * 2 + mci:b * 2 + mci + 1])
                    else:
                        nc.vector.tensor_scalar(
                            dst, src,
                            maskp[:msz, b * 2 + mci:b * 2 + mci + 1],
                            None, op0=AOT.mult)
                vx[(b, mci)] = vt

            def qkproj(j, chi):
                t0 = chi * 392
                ps = pp.tile([128, 392], f32, tag="pj", bufs=3, name="psq")
                for c in range(6):
                    nc.tensor.matmul(
                        ps[:, :],
                        wq[:, _qk_col(c, j):_qk_col(c, j) + 128],
                        xt_ap(c, t0, 392),
                        start=(c == 0), stop=(c == 5),
                    )
                qt = p_qk.tile([128, 392], bf16, tag="qk", bufs=48,
                               name=f"qk{j}_{chi}")
                nc.any.tensor_scalar_add(qt[:, :], ps[:, :],
                                         pars[:, 16 + j:17 + j])
                qk[j][chi] = qt

            def att_s(b, jq):
                tb = (b % 2) * N
                chb = b // 2
                # the two heads' s-matmuls target disjoint 64-row PE tiles
                # (base partitions 0/64) - interleave them so T0/T8 overlap
                ps_ss = [pp.tile([128, 2 * N], f32, tag="ps", bufs=3,
                                 name="pss") for _ in range(2)]
                for hi in range(2):
                    po = hi * 64
                    qAP = qk[jq][chb][po:po + 64, tb:tb + N]
                    for mci, (mc, msz) in enumerate(MCH):
                        kAP = qk[6 + jq][chb][po:po + 64, tb + mc:tb + mc + msz]
                        nc.tensor.matmul(ps_ss[hi][:msz, mci * N:(mci + 1) * N],
                                         kAP, qAP, start=True, stop=True)
                pts = []
                for hi in range(2):
                    h = 2 * jq + hi
                    p0 = p_sm.tile([128, 2 * N], bf16, tag="p0", bufs=8,
                                   name="p0")
                    nc.scalar.activation(p0[:98, :], ps_ss[hi][:98, :], AF.Exp)
                    pt = p_sm.tile([128, 2 * N], bf16, tag="pt", bufs=8,
                                   name="pt")
                    nc.vector.tensor_mul(
                        pt[:98, :], p0[:98, :],
                        bias_t[:98, (h * 2) * N:(h * 2 + 2) * N])
                    pts.append(pt)
                return pts

            def att_o(b, jq, pts):
                ps_o = pp.tile([65, 2 * N], f32, tag="pso", bufs=2, name="pso")
                for hi in range(2):
                    h = 2 * jq + hi
                    for mci, (mc, msz) in enumerate(MCH):
                        vsl = vx[(b, mci)][0:msz, h * 65:h * 65 + 65]
                        nc.tensor.matmul(
                            ps_o[:, hi * N:(hi + 1) * N], vsl,
                            pts[hi][:msz, mci * N:(mci + 1) * N],
                            start=(mci == 0), stop=(mci == 1))
                return ps_o

            def att_n(b, jq, ps_o):
                bp, bi = b // 2, b % 2
                if bp not in cc:
                    cc[bp] = [p_cc.tile([128, 2 * N], bf16, tag="cc", bufs=18,
                                        name=f"cc{bp}_{j}") for j in range(6)]
                den = p_sm.tile([1, 2 * N], f32, tag="den", bufs=8, name="den")
                nc.any.tensor_copy(den[:, :], ps_o[64:65, :])
                rec = p_sm.tile([1, 2 * N], f32, tag="rec", bufs=8, name="rec")
                nc.vector.reciprocal_approx_fast(rec[:, :], den[:, :])
                rb = p_sm.tile([64, 2 * N], f32, tag="rb", bufs=8, name="rb")
                nc.gpsimd.partition_broadcast(rb[:, :], rec[:, :])
                for hi in range(2):
                    nc.vector.tensor_mul(
                        cc[bp][jq][hi * 64:hi * 64 + 64, bi * N:(bi + 1) * N],
                        ps_o[0:64, hi * N:(hi + 1) * N],
                        rb[:, hi * N:(hi + 1) * N])

            def proj_unit(bp, o2, bi_list=(0, 1)):
                w = len(bi_list) * N
                c0 = bi_list[0] * N
                ps = pp.tile([128, 392], f32, tag="pj", bufs=3, name="psp")
                for c2 in range(6):
                    nc.tensor.matmul(ps[:, :w],
                                     wp[:, c2 * 768 + o2 * 128:
                                        c2 * 768 + (o2 + 1) * 128],
                                     cc[bp][c2][:, c0:c0 + w],
                                     start=(c2 == 0), stop=(c2 == 5))
                ot = p_ot.tile([128, 2 * N], bf16, tag="ot", bufs=4, name="ot")
                nc.any.tensor_scalar_add(ot[:, :w], ps[:, :w],
                                         pars[:, 28 + o2:29 + o2])
                nc.sync.dma_start(
                    out_d[o2 * 128:(o2 + 1) * 128,
                          (2 * bp) * N + c0:(2 * bp) * N + c0 + w],
                    ot[:, :w])

            # ---- deadline-driven software pipeline ----
            # One attention unit (b, jq) per step for the DMA-gated first 36
            # units, then 2/step for the last two batches so the projection
            # drain overlaps dense attention work (keeps HAM warm at the tail)
            units = [(b, jq) for b in range(NB) for jq in range(6)]
            SKO, SKN = 2, 3
            ACC_AT = 36

            def step_of(u):
                return u if u < ACC_AT else ACC_AT + (u - ACC_AT + 1) // 2

            JORD = [0, 6, 1, 7, 2, 8, 3, 9, 4, 10, 5, 11]
            fillers = []  # (deadline_step, fn)
            # chunk 0 is DMA-gated (spread over the first steps); later chunks
            # are emitted just-in-time so filler matmuls densify the whole
            # span (a front-loaded PE leaves the tail sparse and HAM
            # re-throttles the clock to 1.2 GHz)
            for ch in range(4):
                for oi, j in enumerate(JORD):
                    dl = (oi // 2 if ch == 0
                          else step_of(12 * ch + oi // 2) - 2)
                    fillers.append((dl, (lambda j=j, c=ch: qkproj(j, c))))
            for b in range(NB):
                dl = max(0, step_of(b * 6 + SKO) - 1)
                for mci in range(2):
                    fillers.append((dl, (lambda b=b, m=mci: vproj(b, m))))
            fillers.sort(key=lambda f: f[0])
            fi = 0

            pj_q = []
            late_q = []
            stage = {}
            si = 0
            se = 0   # att_s emitted up to this unit (runs ahead for pairing)
            PAIR_AT = 12
            n_units = len(units)
            n_steps = step_of(n_units - 1) + SKN + 5
            TAIL_AT = 40
            for i in range(n_steps):
                while fi < len(fillers) and fillers[fi][0] <= i:
                    fillers[fi][1]()
                    fi += 1
                if i >= TAIL_AT and late_q:
                    pj_q.extend(late_q)
                    late_q = []
                for _ in range(2):
                    if pj_q:
                        proj_unit(*pj_q.pop(0))
                quota = 2 if si >= ACC_AT else 1
                sis = list(range(si, si + quota))
                for u in sis:
                    if 0 <= u - SKO < n_units:
                        b, jq = units[u - SKO]
                        stage[(u - SKO, 'o')] = att_o(
                            b, jq, stage.pop((u - SKO, 's')))
                # att_s pairing: in [PAIR_AT, ACC_AT) emit two units' s-blocks
                # together every other step - halves 128<->64-row PE mode
                # transitions without changing the 1-unit/step chain pacing
                if si < PAIR_AT:
                    s_target = si + 1
                elif si < ACC_AT:
                    s_target = min(ACC_AT,
                                   si + (2 if (si - PAIR_AT) % 2 == 0 else 1))
                else:
                    s_target = si + quota
                while se < s_target and se < n_units:
                    b, jq = units[se]
                    stage[(se, 's')] = att_s(b, jq)
                    se += 1
                for u in sis:
                    if 0 <= u - SKN < n_units:
                        b, jq = units[u - SKN]
                        att_n(b, jq, stage.pop((u - SKN, 'o')))
                        if jq == 5:
                            bp, bi = b // 2, b % 2
                            if bp == NB // 2 - 1:
                                pj_q.extend((bp, o2, (bi,))
                                            for o2 in range(6))
                            elif bi == 1:
                                ready = [(bp, o2, (0, 1))
                                         for o2 in range(6)]
                                if bp == 2:
                                    # hold 4 units back for the tail: dense
                                    # full-width PE work that keeps HAM warm
                                    # while the last att batch drains
                                    pj_q.extend(ready[:2])
                                    late_q.extend(ready[2:])
                                else:
                                    pj_q.extend(ready)
                si += quota
            while fi < len(fillers):
                fillers[fi][1]()
                fi += 1
            for args in pj_q + late_q:
                proj_unit(*args)

    nc.finalize()
    return nc


def _prep_in_maps(x, qkv_w, qkv_b, proj_w, proj_b, rpe_table, rpe_index, mask):
    x = np.asarray(x, np.float32)
    qkv_w = np.asarray(qkv_w, np.float32)
    qkv_b = np.asarray(qkv_b, np.float32)
    proj_w = np.asarray(proj_w, np.float32)
    proj_b = np.asarray(proj_b, np.float32)
    rpe_table = np.asarray(rpe_table, np.float32)
    rpe_index = np.asarray(rpe_index)
    mask = np.asarray(mask)

    wqkv = qkv_w.T.copy()              # [C, 3C]
    wqkv[:, :C] *= SCALE               # fold q scaling
    wqkv = wqkv.astype(BF16)
    # assemble the wave-ordered weight blob [128, WQ_COLS] per c-chunk rows
    wq_img = np.zeros((128, WQ_COLS), BF16)
    for c in range(6):
        rows = wqkv[c * 128:(c + 1) * 128]
        for j in range(12):
            col = _qk_col(c, j)
            src = j * 128 if j < 6 else C + (j - 6) * 128
            wq_img[:, col:col + 128] = rows[:, src:src + 128]
        wq_img[:, _v_col(c):_v_col(c) + 768] = rows[:, 2 * C:3 * C]

    wp_img = np.zeros((128, 4608), BF16)
    wproj = proj_w.T.astype(BF16)      # [C, C]
    for c2 in range(6):
        wp_img[:, c2 * 768:(c2 + 1) * 768] = wproj[c2 * 128:(c2 + 1) * 128]

    qkb_full = qkv_b.copy()
    qkb_full[:C] *= SCALE
    qkb = qkb_full[:2 * C].reshape(12, 128).T.astype(np.float32)
    # v bias folded here: softmax rows sum to 1, so +vb before proj is exact
    pjb_full = proj_b + proj_w @ qkv_b[2 * C:]
    pjb = pjb_full.reshape(6, 128).T.astype(np.float32)

    # relative position bias, transposed per head, exponentiated (applied
    # multiplicatively after exp): biasT[p, (h,mc,n)] = exp(bias[h,n,m])
    bias_hnm = rpe_table[rpe_index].reshape(N, N, H).transpose(2, 0, 1)  # [H,n,m]
    bT = np.zeros((H, 2, 128, N), np.float32)
    bT[:, 0, :98, :] = bias_hnm.transpose(0, 2, 1)[:, 0:98, :]
    bT[:, 1, :98, :] = bias_hnm.transpose(0, 2, 1)[:, 98:196, :]
    biasT = np.ascontiguousarray(
        np.exp(bT.transpose(2, 0, 1, 3).reshape(128, H * 2 * N))).astype(BF16)

    in_maps = []
    for i in range(NCORES):
        xs = x[i * NB:(i + 1) * NB].reshape(TOK, C)
        xtT = xs.T.astype(BF16)        # [C, TOK]
        # xt blob: [128, chi*2352 + c*392 + tl]
        xt_img = np.ascontiguousarray(
            xtT.reshape(6, 128, 4, 392).transpose(1, 2, 0, 3).reshape(128, 9408))
        pars = np.zeros((128, 34), np.float32)
        msk = mask[i * NB:(i + 1) * NB]
        for b in range(NB):
            for mci, (mc, msz) in enumerate(MCH):
                col = np.where(msk[b, mc:mc + msz], 1.0, 0.0)
                pars[:msz, b * 2 + mci] = col
        pars[:, 16:28] = qkb
        pars[:, 28:34] = pjb
        in_maps.append({
            "xt": xt_img, "wq": np.ascontiguousarray(wq_img),
            "wp": np.ascontiguousarray(wp_img), "biasT": biasT,
            "pars": np.ascontiguousarray(pars),
        })
    return in_maps


def _run(in_maps, trace=False, tmpdir=None):
    import sys, types
    # antenv.axon_hooks is absent on this image; rebuild the NTFF hook shim
    if trace and 'antenv.axon_hooks' not in sys.modules:
        try:
            import trn_agent_boot.trn_boot as tb
            hook = tb._ntff_profile_via_ctypes('/opt/axon/libaxon_pjrt.so')
            mod = types.ModuleType('antenv.axon_hooks')
            mod.get_axon_ntff_profile_hook = lambda: hook
            import antenv
            antenv.axon_hooks = mod
            sys.modules['antenv.axon_hooks'] = mod
            import concourse.bass_utils as bu
            bu.upload_artifacts = lambda d: d
        except Exception:
            trace = False
    from concourse.bass_utils import run_bass_kernel_spmd
    if 'nc' not in _cache:
        _cache['nc'] = _build_nc()
    return run_bass_kernel_spmd(_cache['nc'], in_maps, list(range(NCORES)),
                                trace=trace, tmpdir=tmpdir)


def kernel(x, qkv_w, qkv_b, proj_w, proj_b, rpe_table, rpe_index, mask):
    in_maps = _prep_in_maps(x, qkv_w, qkv_b, proj_w, proj_b, rpe_table,
                            rpe_index, mask)
    res = _run(in_maps, trace=False)
    out = np.empty((B, N, C), np.float32)
    for i in range(NCORES):
        oc = np.asarray(res.results[i]["out"], np.float32)   # [C, TOK]
        out[i * NB:(i + 1) * NB] = oc.T.reshape(NB, N, C)
    return out
